# revision 5
# baseline (speedup 1.0000x reference)
"""Trainium2 Bass kernel: sparse (top-k) causal attention, data-parallel over batch.

Reference semantics (B=32, H=8, S=512, D=64, k_index=5):
  S_raw = (Q @ K^T) / sqrt(d_k), causal-masked with -inf
  P     = softmax(S_raw)                       (first softmax, rows)
  rows >= k_index: keep only P >= (k_index-th largest of row), else -inf
  W     = softmax(P')                          (second softmax)
  W[row 0] = 0
  out   = W @ V

Key identities used on-chip (per row):
  - first softmax needs no max-subtraction (scores ~ N(0,1), exp safe)
  - top-k threshold in the exp-domain: thr_e = k-th largest of E=exp(s/8);
    P >= thr_p  <=>  E >= thr_e  (softmax is monotone within a row)
  - second softmax: W_j = exp(E_j * (1/Z)) * [E_j >= thr_e], normalized by Z2;
    for rows < k_index all entries pass (thr := -1), masked cols have E=0 so
    exp(0)=1 which matches the reference's exp(p=0)=1 uniform tail; the tail
    columns beyond the causal tile (cols >= 128) contribute (S-128) ones to Z2
    and a rank-1 ones @ V term to the output.
  - row 0 zeroing is applied by forcing its 1/Z2 scale to 0.

Sharding: batch 32 -> 4 per core across 8 cores; each (b,h) is independent.
Host pre-transposes Q,K to [.., D, S] so the PE gets d-major operands, and
pre-casts V to bf16 (second matmul runs bf16 with f32 PSUM accumulate).
"""

import math

import numpy as np
import ml_dtypes

import concourse.bass as bass
import concourse.bacc as bacc
import concourse.mybir as mybir
import concourse.tile as tile
from concourse.bass_utils import run_bass_kernel_spmd
from concourse.masks import make_causal_mask, make_identity

N_CORES = 8
F32 = mybir.dt.float32
BF16 = mybir.dt.bfloat16

# test.py hooks
TRACE = False
LAST_RESULT = None
BH_OVERRIDE = None  # dev only: limit (b,h) pairs per core

_NC_CACHE = {}


def _build(bh_count: int, S: int, D: int, d_k: int, k_index: int) -> bass.Bass:
    P = 128
    NT = S // P
    KI = k_index
    NEG = -1.0e5
    scale = 1.0 / math.sqrt(float(d_k))
    assert 1 <= KI <= 8 and S % P == 0 and D <= P

    nc = bacc.Bacc("TRN2", target_bir_lowering=False, debug=False)
    qt = nc.declare_dram_parameter("qt", [bh_count, D, S], F32, isOutput=False)
    kt = nc.declare_dram_parameter("kt", [bh_count, D, S], F32, isOutput=False)
    vb = nc.declare_dram_parameter("vb", [bh_count, S, D], BF16, isOutput=False)
    out = nc.declare_dram_parameter("out", [bh_count, S, D], F32, isOutput=True)

    with tile.TileContext(nc) as tc:
        with (
            tc.tile_pool(name="const", bufs=1) as cpool,
            tc.tile_pool(name="inp", bufs=3) as ipool,
            tc.tile_pool(name="big", bufs=3) as bpool,
            tc.tile_pool(name="wbuf", bufs=3) as wpool,
            tc.tile_pool(name="wt", bufs=6) as wtpool,
            tc.tile_pool(name="stat", bufs=8) as spool,
            tc.tile_pool(name="obuf", bufs=4) as opool,
            tc.tile_pool(name="ps_s", bufs=2, space="PSUM") as ps_s,
            tc.tile_pool(name="ps_o", bufs=4, space="PSUM") as ps_o,
        ):
            # constants
            mask_f = cpool.tile([P, P], F32)
            make_causal_mask(nc, mask_f[:, :], mask_val=NEG)
            mask_b = cpool.tile([P, P], BF16)
            nc.vector.tensor_copy(mask_b[:, :], mask_f[:, :])
            ident_f = cpool.tile([P, P], F32)
            make_identity(nc, ident_f[:, :])
            ident_b = cpool.tile([P, P], BF16)
            nc.vector.tensor_copy(ident_b[:, :], ident_f[:, :])
            ones_k = cpool.tile([P, KI], BF16)
            nc.vector.memset(ones_k[:, :], 1.0)

            for bh in range(bh_count):
                qt_s = ipool.tile([D, S], F32, tag="qt")
                nc.sync.dma_start(qt_s[:, :], qt[bh])
                kt_s = ipool.tile([D, S], F32, tag="kt")
                nc.sync.dma_start(kt_s[:, :], kt[bh])
                v_s = ipool.tile([P, NT, D], BF16, tag="v")
                nc.sync.dma_start(
                    v_s[:, :, :], vb[bh].rearrange("(c p) d -> p c d", p=P)
                )

                for t in range(NT):
                    C = P * (t + 1)
                    # scores (PSUM), causal mask added to the diagonal block
                    # via a bf16 identity @ mask matmul accumulation
                    s_ps = ps_s.tile([P, S], F32, tag="s")
                    nc.tensor.matmul(
                        s_ps[:, :C],
                        lhsT=qt_s[:, bass.ts(t, P)],
                        rhs=kt_s[:, :C],
                        start=True,
                        stop=False,
                    )
                    nc.tensor.matmul(
                        s_ps[:, bass.ts(t, P)],
                        lhsT=ident_b[:, :],
                        rhs=mask_b[:, :],
                        start=False,
                        stop=True,
                    )

                    # E = exp(s/sqrt(d_k)); Z = row-sum(E) accumulated for free
                    e_s = bpool.tile([P, S], F32, tag="e")
                    z = spool.tile([P, 1], F32, tag="z")
                    nc.scalar.activation(
                        e_s[:, :C],
                        s_ps[:, :C],
                        mybir.ActivationFunctionType.Exp,
                        scale=scale,
                        accum_out=z[:, :],
                    )

                    # top-8 per row -> threshold is col k_index-1
                    top8 = spool.tile([P, 8], F32, tag="top8")
                    nc.vector.max(out=top8[:, :], in_=e_s[:, :C])
                    if t == 0:
                        # rows < k_index keep everything
                        nc.vector.memset(top8[0:KI, KI - 1 : KI], -1.0)

                    rz = spool.tile([P, 1], F32, tag="rz")
                    nc.vector.reciprocal(rz[:, :], z[:, :])

                    # U = exp(E * (1/Z)) = exp(P)
                    u_s = bpool.tile([P, S], F32, tag="u")
                    nc.scalar.activation(
                        u_s[:, :C],
                        e_s[:, :C],
                        mybir.ActivationFunctionType.Exp,
                        scale=rz[:, 0:1],
                    )

                    # W = (E >= thr) * U   (bf16), Z2 = row-sum(W)
                    w_s = wpool.tile([P, S], BF16, tag="w")
                    z2 = spool.tile([P, 1], F32, tag="z2")
                    nc.vector.scalar_tensor_tensor(
                        out=w_s[:, :C],
                        in0=e_s[:, :C],
                        scalar=top8[:, KI - 1 : KI],
                        in1=u_s[:, :C],
                        op0=mybir.AluOpType.is_ge,
                        op1=mybir.AluOpType.mult,
                        accum_out=z2[:, :],
                    )
                    if t == 0:
                        # rows < k_index: uniform tail cols [P, S) add S-P ones
                        nc.vector.tensor_scalar_add(
                            z2[0:KI, :], z2[0:KI, :], float(S - P)
                        )

                    rz2 = spool.tile([P, 1], F32, tag="rz2")
                    nc.vector.reciprocal(rz2[:, :], z2[:, :])
                    if t == 0:
                        # zero_pad: row 0 gets all-zero attention
                        nc.vector.memset(rz2[0:1, :], 0.0)

                    # O = W @ V via DMA-transposed bf16 W chunks
                    o_ps = ps_o.tile([P, D], F32, tag="o")
                    for c in range(t + 1):
                        wt_s = wtpool.tile([P, P], BF16, tag="wt")
                        nc.sync.dma_start(
                            wt_s[:, :], w_s[:, bass.ts(c, P)], transpose=True
                        )
                        nc.tensor.matmul(
                            o_ps[:, :],
                            lhsT=wt_s[:, :],
                            rhs=v_s[:, c, :],
                            start=(c == 0),
                            stop=(c == t and t > 0),
                        )
                    if t == 0:
                        # rows < k_index: += ones @ V over tail chunks
                        for c in range(1, NT):
                            nc.tensor.matmul(
                                o_ps[0:KI, :],
                                lhsT=ones_k[:, 0:KI],
                                rhs=v_s[:, c, :],
                                start=False,
                                stop=(c == NT - 1),
                            )

                    o_s = opool.tile([P, D], F32, tag="o_s")
                    nc.vector.tensor_scalar(
                        out=o_s[:, :],
                        in0=o_ps[:, :],
                        scalar1=rz2[:, 0:1],
                        scalar2=None,
                        op0=mybir.AluOpType.mult,
                    )
                    nc.sync.dma_start(out[bh, bass.ts(t, P), :], o_s[:, :])
    nc.compile()
    return nc


def _get_nc(bh_count, S, D, d_k, k_index):
    key = (bh_count, S, D, d_k, k_index)
    if key not in _NC_CACHE:
        _NC_CACHE[key] = _build(bh_count, S, D, d_k, k_index)
    return _NC_CACHE[key]


def kernel(q, k, v, mask=None, d_k=None, k_index=None, **_unused):
    global LAST_RESULT
    q = np.asarray(q, dtype=np.float32)
    k = np.asarray(k, dtype=np.float32)
    v = np.asarray(v, dtype=np.float32)
    B, H, S, D = q.shape
    d_k = int(d_k) if d_k is not None else D
    k_index = int(k_index) if k_index is not None else 5

    bpc = B // N_CORES
    bh_full = bpc * H
    bh_count = BH_OVERRIDE or bh_full

    qt = np.ascontiguousarray(np.transpose(q, (0, 1, 3, 2)))  # [B,H,D,S]
    ktr = np.ascontiguousarray(np.transpose(k, (0, 1, 3, 2)))
    vb = np.ascontiguousarray(v.astype(ml_dtypes.bfloat16))

    nc = _get_nc(bh_count, S, D, d_k, k_index)

    in_maps = []
    for i in range(N_CORES):
        sl = slice(i * bpc, (i + 1) * bpc)
        in_maps.append(
            {
                "qt": qt[sl].reshape(bh_full, D, S)[:bh_count],
                "kt": ktr[sl].reshape(bh_full, D, S)[:bh_count],
                "vb": vb[sl].reshape(bh_full, S, D)[:bh_count],
            }
        )

    res = run_bass_kernel_spmd(
        nc, in_maps, core_ids=list(range(N_CORES)), trace=TRACE
    )
    LAST_RESULT = res

    outs = [
        np.asarray(res.results[i]["out"], dtype=np.float32) for i in range(N_CORES)
    ]
    if bh_count != bh_full:
        outs = [
            np.concatenate(
                [o, np.zeros((bh_full - bh_count, S, D), np.float32)], axis=0
            )
            for o in outs
        ]
    return np.concatenate([o.reshape(bpc, H, S, D) for o in outs], axis=0)


# revision 6
# speedup vs baseline: 1.5654x; 1.5654x over previous
"""Trainium2 Bass kernel: sparse (top-k) causal attention, data-parallel over batch.

Reference semantics (B=32, H=8, S=512, D=64, k_index=5):
  S_raw = (Q @ K^T) / sqrt(d_k), causal-masked
  P     = softmax(S_raw)
  rows >= k_index: keep only P >= (k_index-th largest of row)
  W     = softmax(P');  W[row 0] = 0;  out = W @ V

On-chip identities (per row):
  - no max-subtraction needed (scores ~ N(0,1))
  - top-k threshold via DVE top-8 in the exp-domain (softmax is monotone)
  - W = (E >= thr) * exp(E/Z) via one fused scalar_tensor_tensor with
    accumulated row-sum Z2; rows < k_index pass everything (thr=-1) and the
    causal-masked cols contribute exp(0)=1, matching the reference; their
    uniform tail beyond the causal tile adds (S-128) to Z2 and a rank-1
    ones @ V term to the output; row 0 is zeroed via its 1/Z2 scale.

Sharding: batch 32 -> 4 per core across 8 cores; each (b,h) independent.
Host packs Q,K pre-transposed into one [.., D, 2S] tensor and V as bf16.
"""

import math

import numpy as np
import ml_dtypes

import concourse.bass as bass
import concourse.bacc as bacc
import concourse.mybir as mybir
import concourse.tile as tile
from concourse.bass_utils import run_bass_kernel_spmd
from concourse.masks import make_causal_mask, make_identity

N_CORES = 8
F32 = mybir.dt.float32
BF16 = mybir.dt.bfloat16

# test.py hooks
TRACE = False
LAST_RESULT = None
BH_OVERRIDE = None  # dev only: limit (b,h) pairs per core
QK_DTYPE = mybir.dt.float32r  # matmul1 operand dtype (float32r: full PE rate)

_NC_CACHE = {}


def _build(bh_count: int, S: int, D: int, d_k: int, k_index: int) -> bass.Bass:
    P = 128
    NT = S // P
    KI = k_index
    NEG = -1.0e5
    scale = 1.0 / math.sqrt(float(d_k))
    assert 1 <= KI <= 8 and S % P == 0 and D <= P

    nc = bacc.Bacc("TRN2", target_bir_lowering=False, debug=False)
    qkt = nc.declare_dram_parameter("qkt", [bh_count, D, 2 * S], QK_DTYPE, isOutput=False)
    vb = nc.declare_dram_parameter("vb", [bh_count, S, D], BF16, isOutput=False)
    out = nc.declare_dram_parameter("out", [bh_count, S, D], F32, isOutput=True)

    with tile.TileContext(nc) as tc:
        with (
            tc.tile_pool(name="const", bufs=1) as cpool,
            tc.tile_pool(name="inp", bufs=4) as ipool,
            tc.tile_pool(name="big", bufs=4) as bpool,
            tc.tile_pool(name="wbuf", bufs=4) as wpool,
            tc.tile_pool(name="wt", bufs=6) as wtpool,
            tc.tile_pool(name="stat", bufs=16) as spool,
            tc.tile_pool(name="obuf", bufs=3) as opool,
            tc.tile_pool(name="ps_s", bufs=3, space="PSUM") as ps_s,
            tc.tile_pool(name="ps_o", bufs=4, space="PSUM") as ps_o,
        ):
            # constants
            mask_f = cpool.tile([P, P], F32)
            make_causal_mask(nc, mask_f[:, :], mask_val=NEG)
            mask_b = cpool.tile([P, P], BF16)
            nc.vector.tensor_copy(mask_b[:, :], mask_f[:, :])
            ident_f = cpool.tile([P, P], F32)
            make_identity(nc, ident_f[:, :])
            ident_b = cpool.tile([P, P], BF16)
            nc.vector.tensor_copy(ident_b[:, :], ident_f[:, :])
            ones_k = cpool.tile([P, KI], BF16)
            nc.vector.memset(ones_k[:, :], 1.0)

            for bh in range(bh_count):
                qk_s = ipool.tile([D, 2 * S], QK_DTYPE, tag="qk")
                nc.gpsimd.dma_start(qk_s[:, :], qkt[bh])
                v_s = ipool.tile([P, NT, D], BF16, tag="v")
                nc.gpsimd.dma_start(
                    v_s[:, :, :], vb[bh].rearrange("(c p) d -> p c d", p=P)
                )
                o_all = opool.tile([P, NT, D], F32, tag="o_all")

                for t in range(NT):
                    C = P * (t + 1)
                    s_ps = ps_s.tile([P, S], F32, tag="s")
                    nc.tensor.matmul(
                        s_ps[:, :C],
                        lhsT=qk_s[:, bass.ts(t, P)],
                        rhs=qk_s[:, S : S + C],
                        start=True,
                        stop=False,
                    )
                    nc.tensor.matmul(
                        s_ps[:, bass.ts(t, P)],
                        lhsT=ident_b[:, :],
                        rhs=mask_b[:, :],
                        start=False,
                        stop=True,
                    )

                    # E = exp(s/sqrt(d_k)); Z = row-sum(E) via accumulate
                    e_s = bpool.tile([P, S], F32, tag="e")
                    z = spool.tile([P, 1], F32, tag="z")
                    nc.scalar.activation(
                        e_s[:, :C],
                        s_ps[:, :C],
                        mybir.ActivationFunctionType.Exp,
                        scale=scale,
                        accum_out=z[:, :],
                    )

                    top8 = spool.tile([P, 8], F32, tag="top8")
                    nc.vector.max(out=top8[:, :], in_=e_s[:, :C])
                    if t == 0:
                        nc.vector.memset(top8[0:KI, KI - 1 : KI], -1.0)

                    rz = spool.tile([P, 1], F32, tag="rz")
                    nc.vector.reciprocal(rz[:, :], z[:, :])

                    # U = exp(E/Z) = exp(P)
                    u_s = bpool.tile([P, S], F32, tag="u")
                    nc.scalar.activation(
                        u_s[:, :C],
                        e_s[:, :C],
                        mybir.ActivationFunctionType.Exp,
                        scale=rz[:, 0:1],
                    )

                    # W = (E >= thr) * U  (bf16), Z2 = row-sum(W)
                    w_s = wpool.tile([P, S], BF16, tag="w")
                    z2 = spool.tile([P, 1], F32, tag="z2")
                    nc.vector.scalar_tensor_tensor(
                        out=w_s[:, :C],
                        in0=e_s[:, :C],
                        scalar=top8[:, KI - 1 : KI],
                        in1=u_s[:, :C],
                        op0=mybir.AluOpType.is_ge,
                        op1=mybir.AluOpType.mult,
                        accum_out=z2[:, :],
                    )
                    if t == 0:
                        nc.vector.tensor_scalar_add(
                            z2[0:KI, :], z2[0:KI, :], float(S - P)
                        )

                    rz2 = spool.tile([P, 1], F32, tag="rz2")
                    nc.vector.reciprocal(rz2[:, :], z2[:, :])
                    if t == 0:
                        nc.vector.memset(rz2[0:1, :], 0.0)

                    # W^T chunks via one 3D-output xbar DMA transpose
                    wt_s = wtpool.tile([P, NT, P], BF16, tag="wt")
                    nc.sync.dma_start(
                        wt_s[:, 0 : t + 1, :], w_s[:, :C], transpose=True
                    )

                    o_ps = ps_o.tile([P, D], F32, tag="o")
                    for c in range(t + 1):
                        nc.tensor.matmul(
                            o_ps[:, :],
                            lhsT=wt_s[:, c, :],
                            rhs=v_s[:, c, :],
                            start=(c == 0),
                            stop=(c == t and t > 0),
                        )
                    if t == 0:
                        for c in range(1, NT):
                            nc.tensor.matmul(
                                o_ps[0:KI, :],
                                lhsT=ones_k[:, 0:KI],
                                rhs=v_s[:, c, :],
                                start=False,
                                stop=(c == NT - 1),
                            )

                    nc.vector.tensor_scalar(
                        out=o_all[:, t, :],
                        in0=o_ps[:, :],
                        scalar1=rz2[:, 0:1],
                        scalar2=None,
                        op0=mybir.AluOpType.mult,
                    )
                nc.gpsimd.dma_start(
                    out[bh].rearrange("(c p) d -> p c d", p=P), o_all[:, :, :]
                )
    nc.compile()
    return nc


def _get_nc(bh_count, S, D, d_k, k_index):
    key = (bh_count, S, D, d_k, k_index, str(QK_DTYPE))
    if key not in _NC_CACHE:
        _NC_CACHE[key] = _build(bh_count, S, D, d_k, k_index)
    return _NC_CACHE[key]


def kernel(q, k, v, mask=None, d_k=None, k_index=None, **_unused):
    global LAST_RESULT
    q = np.asarray(q, dtype=np.float32)
    k = np.asarray(k, dtype=np.float32)
    v = np.asarray(v, dtype=np.float32)
    B, H, S, D = q.shape
    d_k = int(d_k) if d_k is not None else D
    k_index = int(k_index) if k_index is not None else 5

    bpc = B // N_CORES
    bh_full = bpc * H
    bh_count = BH_OVERRIDE or bh_full

    qkt = np.concatenate(
        [np.transpose(q, (0, 1, 3, 2)), np.transpose(k, (0, 1, 3, 2))], axis=3
    )  # [B, H, D, 2S]
    qkt = np.ascontiguousarray(qkt)
    vb = np.ascontiguousarray(v.astype(ml_dtypes.bfloat16))

    nc = _get_nc(bh_count, S, D, d_k, k_index)

    in_maps = []
    for i in range(N_CORES):
        sl = slice(i * bpc, (i + 1) * bpc)
        in_maps.append(
            {
                "qkt": qkt[sl].reshape(bh_full, D, 2 * S)[:bh_count],
                "vb": vb[sl].reshape(bh_full, S, D)[:bh_count],
            }
        )

    res = run_bass_kernel_spmd(
        nc, in_maps, core_ids=list(range(N_CORES)), trace=TRACE
    )
    LAST_RESULT = res

    outs = [
        np.asarray(res.results[i]["out"], dtype=np.float32) for i in range(N_CORES)
    ]
    if bh_count != bh_full:
        outs = [
            np.concatenate(
                [o, np.zeros((bh_full - bh_count, S, D), np.float32)], axis=0
            )
            for o in outs
        ]
    return np.concatenate([o.reshape(bpc, H, S, D) for o in outs], axis=0)


# revision 7
# speedup vs baseline: 1.5971x; 1.0203x over previous
"""Trainium2 Bass kernel: sparse (top-k) causal attention, data-parallel over batch.

Reference semantics (B=32, H=8, S=512, D=64, k_index=5):
  S_raw = (Q @ K^T) / sqrt(d_k), causal-masked
  P     = softmax(S_raw)
  rows >= k_index: keep only P >= (k_index-th largest of row)
  W     = softmax(P');  W[row 0] = 0;  out = W @ V

On-chip identities (per row):
  - no max-subtraction needed (scores ~ N(0,1))
  - top-k threshold via DVE top-8 in the exp-domain (softmax is monotone)
  - W = (E >= thr) * exp(E/Z) via one fused scalar_tensor_tensor with
    accumulated row-sum Z2; rows < k_index pass everything (thr=-1) and the
    causal-masked cols contribute exp(0)=1, matching the reference; their
    uniform tail beyond the causal tile adds (S-128) to Z2 and a rank-1
    ones @ V term to the output; row 0 is zeroed via its 1/Z2 scale.

Sharding: batch 32 -> 4 per core across 8 cores; each (b,h) independent.
Host packs Q,K pre-transposed into one [.., D, 2S] tensor and V as bf16.
"""

import math

import numpy as np
import ml_dtypes

import concourse.bass as bass
import concourse.bacc as bacc
import concourse.mybir as mybir
import concourse.tile as tile
from concourse.bass_utils import run_bass_kernel_spmd
from concourse.masks import make_causal_mask, make_identity

N_CORES = 8
F32 = mybir.dt.float32
BF16 = mybir.dt.bfloat16

# test.py hooks
TRACE = False
LAST_RESULT = None
BH_OVERRIDE = None  # dev only: limit (b,h) pairs per core
QK_DTYPE = mybir.dt.float32  # matmul1 operand dtype (f32: exact top-k selection)

_NC_CACHE = {}


def _build(bh_count: int, S: int, D: int, d_k: int, k_index: int) -> bass.Bass:
    P = 128
    NT = S // P
    KI = k_index
    NEG = -1.0e5
    scale = 1.0 / math.sqrt(float(d_k))
    assert 1 <= KI <= 8 and S % P == 0 and D <= P

    nc = bacc.Bacc("TRN2", target_bir_lowering=False, debug=False)
    qkt = nc.declare_dram_parameter("qkt", [bh_count, D, 2 * S], QK_DTYPE, isOutput=False)
    vb = nc.declare_dram_parameter("vb", [bh_count, S, D], BF16, isOutput=False)
    out = nc.declare_dram_parameter("out", [bh_count, S, D], F32, isOutput=True)

    with tile.TileContext(nc) as tc:
        with (
            tc.tile_pool(name="const", bufs=1) as cpool,
            tc.tile_pool(name="inp", bufs=4) as ipool,
            tc.tile_pool(name="big", bufs=4) as bpool,
            tc.tile_pool(name="wbuf", bufs=4) as wpool,
            tc.tile_pool(name="wt", bufs=6) as wtpool,
            tc.tile_pool(name="stat", bufs=16) as spool,
            tc.tile_pool(name="obuf", bufs=3) as opool,
            tc.tile_pool(name="ps_s", bufs=3, space="PSUM") as ps_s,
            tc.tile_pool(name="ps_o", bufs=4, space="PSUM") as ps_o,
        ):
            # constants
            mask_f = cpool.tile([P, P], F32)
            make_causal_mask(nc, mask_f[:, :], mask_val=NEG)
            mask_b = cpool.tile([P, P], BF16)
            nc.vector.tensor_copy(mask_b[:, :], mask_f[:, :])
            ident_f = cpool.tile([P, P], F32)
            make_identity(nc, ident_f[:, :])
            ident_b = cpool.tile([P, P], BF16)
            nc.vector.tensor_copy(ident_b[:, :], ident_f[:, :])
            ones_k = cpool.tile([P, KI], BF16)
            nc.vector.memset(ones_k[:, :], 1.0)

            for bh in range(bh_count):
                qk_s = ipool.tile([D, 2 * S], QK_DTYPE, tag="qk")
                nc.gpsimd.dma_start(qk_s[:, :], qkt[bh])
                v_s = ipool.tile([P, NT, D], BF16, tag="v")
                nc.gpsimd.dma_start(
                    v_s[:, :, :], vb[bh].rearrange("(c p) d -> p c d", p=P)
                )
                o_all = opool.tile([P, NT, D], F32, tag="o_all")

                for t in range(NT):
                    C = P * (t + 1)
                    s_ps = ps_s.tile([P, S], F32, tag="s")
                    nc.tensor.matmul(
                        s_ps[:, :C],
                        lhsT=qk_s[:, bass.ts(t, P)],
                        rhs=qk_s[:, S : S + C],
                        start=True,
                        stop=False,
                    )
                    nc.tensor.matmul(
                        s_ps[:, bass.ts(t, P)],
                        lhsT=ident_b[:, :],
                        rhs=mask_b[:, :],
                        start=False,
                        stop=True,
                    )

                    # E = exp(s/sqrt(d_k)); Z = row-sum(E) via accumulate
                    e_s = bpool.tile([P, S], F32, tag="e")
                    z = spool.tile([P, 1], F32, tag="z")
                    nc.scalar.activation(
                        e_s[:, :C],
                        s_ps[:, :C],
                        mybir.ActivationFunctionType.Exp,
                        scale=scale,
                        accum_out=z[:, :],
                    )

                    top8 = spool.tile([P, 8], F32, tag="top8")
                    nc.vector.max(out=top8[:, :], in_=e_s[:, :C])
                    if t == 0:
                        nc.vector.memset(top8[0:KI, KI - 1 : KI], -1.0)

                    rz = spool.tile([P, 1], F32, tag="rz")
                    nc.vector.reciprocal(rz[:, :], z[:, :])

                    # U = exp(E/Z) = exp(P)
                    u_s = bpool.tile([P, S], F32, tag="u")
                    nc.scalar.activation(
                        u_s[:, :C],
                        e_s[:, :C],
                        mybir.ActivationFunctionType.Exp,
                        scale=rz[:, 0:1],
                    )

                    # W = (E >= thr) * U  (bf16), Z2 = row-sum(W)
                    w_s = wpool.tile([P, S], BF16, tag="w")
                    z2 = spool.tile([P, 1], F32, tag="z2")
                    nc.vector.scalar_tensor_tensor(
                        out=w_s[:, :C],
                        in0=e_s[:, :C],
                        scalar=top8[:, KI - 1 : KI],
                        in1=u_s[:, :C],
                        op0=mybir.AluOpType.is_ge,
                        op1=mybir.AluOpType.mult,
                        accum_out=z2[:, :],
                    )
                    if t == 0:
                        nc.vector.tensor_scalar_add(
                            z2[0:KI, :], z2[0:KI, :], float(S - P)
                        )

                    rz2 = spool.tile([P, 1], F32, tag="rz2")
                    nc.vector.reciprocal(rz2[:, :], z2[:, :])
                    if t == 0:
                        nc.vector.memset(rz2[0:1, :], 0.0)

                    # W^T chunks via one 3D-output xbar DMA transpose
                    wt_s = wtpool.tile([P, NT, P], BF16, tag="wt")
                    nc.sync.dma_start(
                        wt_s[:, 0 : t + 1, :], w_s[:, :C], transpose=True
                    )

                    o_ps = ps_o.tile([P, D], F32, tag="o")
                    for c in range(t + 1):
                        nc.tensor.matmul(
                            o_ps[:, :],
                            lhsT=wt_s[:, c, :],
                            rhs=v_s[:, c, :],
                            start=(c == 0),
                            stop=(c == t and t > 0),
                        )
                    if t == 0:
                        for c in range(1, NT):
                            nc.tensor.matmul(
                                o_ps[0:KI, :],
                                lhsT=ones_k[:, 0:KI],
                                rhs=v_s[:, c, :],
                                start=False,
                                stop=(c == NT - 1),
                            )

                    nc.vector.tensor_scalar(
                        out=o_all[:, t, :],
                        in0=o_ps[:, :],
                        scalar1=rz2[:, 0:1],
                        scalar2=None,
                        op0=mybir.AluOpType.mult,
                    )
                nc.gpsimd.dma_start(
                    out[bh].rearrange("(c p) d -> p c d", p=P), o_all[:, :, :]
                )
    nc.compile()
    return nc


def _get_nc(bh_count, S, D, d_k, k_index):
    key = (bh_count, S, D, d_k, k_index, str(QK_DTYPE))
    if key not in _NC_CACHE:
        _NC_CACHE[key] = _build(bh_count, S, D, d_k, k_index)
    return _NC_CACHE[key]


def kernel(q, k, v, mask=None, d_k=None, k_index=None, **_unused):
    global LAST_RESULT
    q = np.asarray(q, dtype=np.float32)
    k = np.asarray(k, dtype=np.float32)
    v = np.asarray(v, dtype=np.float32)
    B, H, S, D = q.shape
    d_k = int(d_k) if d_k is not None else D
    k_index = int(k_index) if k_index is not None else 5

    bpc = B // N_CORES
    bh_full = bpc * H
    bh_count = BH_OVERRIDE or bh_full

    qkt = np.concatenate(
        [np.transpose(q, (0, 1, 3, 2)), np.transpose(k, (0, 1, 3, 2))], axis=3
    )  # [B, H, D, 2S]
    qkt = np.ascontiguousarray(qkt)
    vb = np.ascontiguousarray(v.astype(ml_dtypes.bfloat16))

    nc = _get_nc(bh_count, S, D, d_k, k_index)

    in_maps = []
    for i in range(N_CORES):
        sl = slice(i * bpc, (i + 1) * bpc)
        in_maps.append(
            {
                "qkt": qkt[sl].reshape(bh_full, D, 2 * S)[:bh_count],
                "vb": vb[sl].reshape(bh_full, S, D)[:bh_count],
            }
        )

    res = run_bass_kernel_spmd(
        nc, in_maps, core_ids=list(range(N_CORES)), trace=TRACE
    )
    LAST_RESULT = res

    outs = [
        np.asarray(res.results[i]["out"], dtype=np.float32) for i in range(N_CORES)
    ]
    if bh_count != bh_full:
        outs = [
            np.concatenate(
                [o, np.zeros((bh_full - bh_count, S, D), np.float32)], axis=0
            )
            for o in outs
        ]
    return np.concatenate([o.reshape(bpc, H, S, D) for o in outs], axis=0)


# revision 11
# speedup vs baseline: 1.6747x; 1.0486x over previous
"""Trainium2 Bass kernel: sparse (top-k) causal attention, data-parallel over batch.

Reference semantics (B=32, H=8, S=512, D=64, k_index=5):
  S_raw = (Q @ K^T) / sqrt(d_k), causal-masked
  P     = softmax(S_raw)
  rows >= k_index: keep only P >= (k_index-th largest of row)
  W     = softmax(P');  W[row 0] = 0;  out = W @ V

On-chip identities (per row):
  - no max-subtraction needed (scores ~ N(0,1))
  - top-k threshold via DVE top-8 in the exp-domain (softmax is monotone)
  - W = (E >= thr) * exp(E/Z) via one fused scalar_tensor_tensor with
    accumulated row-sum Z2; rows < k_index pass everything (thr=-1) and the
    causal-masked cols contribute exp(0)=1, matching the reference; their
    uniform tail beyond the causal tile adds (S-128) to Z2 and a rank-1
    ones @ V term to the output; row 0 is zeroed via its 1/Z2 scale.

Sharding: batch 32 -> 4 per core across 8 cores; each (b,h) independent.
Host packs Q,K pre-transposed into one [.., D, 2S] tensor and V as bf16.
"""

import math

import numpy as np
import ml_dtypes

import concourse.bass as bass
import concourse.bacc as bacc
import concourse.mybir as mybir
import concourse.tile as tile
from concourse.bass_utils import run_bass_kernel_spmd
from concourse.masks import make_causal_mask, make_identity

N_CORES = 8
F32 = mybir.dt.float32
BF16 = mybir.dt.bfloat16

# test.py hooks
TRACE = False
LAST_RESULT = None
BH_OVERRIDE = None  # dev only: limit (b,h) pairs per core
QK_DTYPE = mybir.dt.float32  # matmul1 operand dtype (f32: exact top-k selection)

_NC_CACHE = {}


def _build(bh_count: int, S: int, D: int, d_k: int, k_index: int) -> bass.Bass:
    P = 128
    NT = S // P
    KI = k_index
    NEG = -1.0e5
    scale = 1.0 / math.sqrt(float(d_k))
    assert 1 <= KI <= 8 and S % P == 0 and D <= P

    nc = bacc.Bacc("TRN2", target_bir_lowering=False, debug=False)
    qkt = nc.declare_dram_parameter("qkt", [bh_count, D, 2 * S], QK_DTYPE, isOutput=False)
    vb = nc.declare_dram_parameter("vb", [bh_count, S, D], BF16, isOutput=False)
    out = nc.declare_dram_parameter("out", [bh_count, S, D], F32, isOutput=True)

    G = 2  # heads interleaved per group (pipeline width)
    with tile.TileContext(nc) as tc:
        with (
            tc.tile_pool(name="const", bufs=1) as cpool,
            tc.tile_pool(name="inp", bufs=4) as ipool,
            tc.tile_pool(name="big", bufs=6) as bpool,
            tc.tile_pool(name="wbuf", bufs=6) as wpool,
            tc.tile_pool(name="wt", bufs=8) as wtpool,
            tc.tile_pool(name="stat", bufs=24) as spool,
            tc.tile_pool(name="obuf", bufs=4) as opool,
            tc.tile_pool(name="ps_s", bufs=4, space="PSUM") as ps_s,
            tc.tile_pool(name="ps_o", bufs=4, space="PSUM") as ps_o,
        ):
            # constants
            mask_f = cpool.tile([P, P], F32)
            make_causal_mask(nc, mask_f[:, :], mask_val=NEG)
            mask_b = cpool.tile([P, P], BF16)
            nc.vector.tensor_copy(mask_b[:, :], mask_f[:, :])
            ident_f = cpool.tile([P, P], F32)
            make_identity(nc, ident_f[:, :])
            ident_b = cpool.tile([P, P], BF16)
            nc.vector.tensor_copy(ident_b[:, :], ident_f[:, :])
            ones_k = cpool.tile([P, KI], BF16)
            nc.vector.memset(ones_k[:, :], 1.0)

            for g0 in range(0, bh_count, G):
                members = list(range(g0, min(g0 + G, bh_count)))
                qk_m, v_m, o_m = {}, {}, {}
                for bh in members:
                    qk_s = ipool.tile([D, 2 * S], QK_DTYPE, tag=f"qk{bh % G}")
                    nc.gpsimd.dma_start(qk_s[:, :], qkt[bh])
                    v_s = ipool.tile([P, NT, D], BF16, tag=f"v{bh % G}")
                    nc.gpsimd.dma_start(
                        v_s[:, :, :], vb[bh].rearrange("(c p) d -> p c d", p=P)
                    )
                    qk_m[bh], v_m[bh] = qk_s, v_s
                    o_m[bh] = opool.tile(
                        [P, NT, D], F32, tag=f"o_all{bh % G}", name=f"o_all_{bh}"
                    )

                for t in range(NT):
                  for bh in members:
                    qk_s, v_s, o_all = qk_m[bh], v_m[bh], o_m[bh]
                    C = P * (t + 1)
                    s_ps = ps_s.tile([P, S], F32, tag="s")
                    nc.tensor.matmul(
                        s_ps[:, :C],
                        lhsT=qk_s[:, bass.ts(t, P)],
                        rhs=qk_s[:, S : S + C],
                        start=True,
                        stop=False,
                    )
                    nc.tensor.matmul(
                        s_ps[:, bass.ts(t, P)],
                        lhsT=ident_b[:, :],
                        rhs=mask_b[:, :],
                        start=False,
                        stop=True,
                    )

                    # E = exp(s/sqrt(d_k)); Z = row-sum(E) via accumulate
                    e_s = bpool.tile([P, S], F32, tag="e")
                    z = spool.tile([P, 1], F32, tag="z")
                    nc.scalar.activation(
                        e_s[:, :C],
                        s_ps[:, :C],
                        mybir.ActivationFunctionType.Exp,
                        scale=scale,
                        accum_out=z[:, :],
                    )

                    top8 = spool.tile([P, 8], F32, tag="top8")
                    nc.vector.max(out=top8[:, :], in_=e_s[:, :C])
                    if t == 0:
                        nc.vector.memset(top8[0:KI, KI - 1 : KI], -1.0)

                    rz = spool.tile([P, 1], F32, tag="rz")
                    nc.vector.reciprocal(rz[:, :], z[:, :])

                    # U = exp(E/Z) = exp(P)
                    u_s = bpool.tile([P, S], F32, tag="u")
                    nc.scalar.activation(
                        u_s[:, :C],
                        e_s[:, :C],
                        mybir.ActivationFunctionType.Exp,
                        scale=rz[:, 0:1],
                    )

                    # W = (E >= thr) * U  (bf16), Z2 = row-sum(W)
                    w_s = wpool.tile([P, S], BF16, tag="w")
                    z2 = spool.tile([P, 1], F32, tag="z2")
                    nc.vector.scalar_tensor_tensor(
                        out=w_s[:, :C],
                        in0=e_s[:, :C],
                        scalar=top8[:, KI - 1 : KI],
                        in1=u_s[:, :C],
                        op0=mybir.AluOpType.is_ge,
                        op1=mybir.AluOpType.mult,
                        accum_out=z2[:, :],
                    )
                    if t == 0:
                        nc.vector.tensor_scalar_add(
                            z2[0:KI, :], z2[0:KI, :], float(S - P)
                        )

                    rz2 = spool.tile([P, 1], F32, tag="rz2")
                    nc.vector.reciprocal(rz2[:, :], z2[:, :])
                    if t == 0:
                        nc.vector.memset(rz2[0:1, :], 0.0)

                    # W^T chunks via one 3D-output xbar DMA transpose
                    wt_s = wtpool.tile([P, NT, P], BF16, tag="wt")
                    nc.sync.dma_start(
                        wt_s[:, 0 : t + 1, :], w_s[:, :C], transpose=True
                    )

                    o_ps = ps_o.tile([P, D], F32, tag="o")
                    for c in range(t + 1):
                        nc.tensor.matmul(
                            o_ps[:, :],
                            lhsT=wt_s[:, c, :],
                            rhs=v_s[:, c, :],
                            start=(c == 0),
                            stop=(c == t and t > 0),
                        )
                    if t == 0:
                        for c in range(1, NT):
                            nc.tensor.matmul(
                                o_ps[0:KI, :],
                                lhsT=ones_k[:, 0:KI],
                                rhs=v_s[:, c, :],
                                start=False,
                                stop=(c == NT - 1),
                            )

                    nc.vector.tensor_scalar(
                        out=o_all[:, t, :],
                        in0=o_ps[:, :],
                        scalar1=rz2[:, 0:1],
                        scalar2=None,
                        op0=mybir.AluOpType.mult,
                    )
                for bh in members:
                    nc.gpsimd.dma_start(
                        out[bh].rearrange("(c p) d -> p c d", p=P),
                        o_m[bh][:, :, :],
                    )
    nc.compile()
    return nc


def _get_nc(bh_count, S, D, d_k, k_index):
    key = (bh_count, S, D, d_k, k_index, str(QK_DTYPE))
    if key not in _NC_CACHE:
        _NC_CACHE[key] = _build(bh_count, S, D, d_k, k_index)
    return _NC_CACHE[key]


def kernel(q, k, v, mask=None, d_k=None, k_index=None, **_unused):
    global LAST_RESULT
    q = np.asarray(q, dtype=np.float32)
    k = np.asarray(k, dtype=np.float32)
    v = np.asarray(v, dtype=np.float32)
    B, H, S, D = q.shape
    d_k = int(d_k) if d_k is not None else D
    k_index = int(k_index) if k_index is not None else 5

    bpc = B // N_CORES
    bh_full = bpc * H
    bh_count = BH_OVERRIDE or bh_full

    qkt = np.concatenate(
        [np.transpose(q, (0, 1, 3, 2)), np.transpose(k, (0, 1, 3, 2))], axis=3
    )  # [B, H, D, 2S]
    qkt = np.ascontiguousarray(qkt)
    vb = np.ascontiguousarray(v.astype(ml_dtypes.bfloat16))

    nc = _get_nc(bh_count, S, D, d_k, k_index)

    in_maps = []
    for i in range(N_CORES):
        sl = slice(i * bpc, (i + 1) * bpc)
        in_maps.append(
            {
                "qkt": qkt[sl].reshape(bh_full, D, 2 * S)[:bh_count],
                "vb": vb[sl].reshape(bh_full, S, D)[:bh_count],
            }
        )

    res = run_bass_kernel_spmd(
        nc, in_maps, core_ids=list(range(N_CORES)), trace=TRACE
    )
    LAST_RESULT = res

    outs = [
        np.asarray(res.results[i]["out"], dtype=np.float32) for i in range(N_CORES)
    ]
    if bh_count != bh_full:
        outs = [
            np.concatenate(
                [o, np.zeros((bh_full - bh_count, S, D), np.float32)], axis=0
            )
            for o in outs
        ]
    return np.concatenate([o.reshape(bpc, H, S, D) for o in outs], axis=0)
